# revision 11
# baseline (speedup 1.0000x reference)
"""BoundaryDoULoss Trainium2 kernel.

Data-parallel over batch: 16 images are sharded 2-per-core across 8
NeuronCores. Each core computes per-class partial sums (S = region count,
C = boundary count, I = sum(probs * onehot), Z = sum(probs^2)) over its
shard; the host reduces the per-class partial scalars and forms alpha and
the final scalar loss.

Layout per image: [512, 512] -> [128 partitions, 4 rows x 512 cols free].
Vertical neighbor comparisons are free-dim shifts within a partition; the
one row per partition that crosses a partition boundary is handled with two
halo tiles DMA-loaded straight from DRAM (rows 4,8,...,508 and 3,7,...,507).
A sentinel row of 448s (not a class id) feeds the image-top/bottom halo
slots so border rows come out as boundary automatically.

Inputs travel as bf16 (halves HBM traffic; quantizing the logits moves the
final loss by ~7e-7 relative - per-pixel rounding noise cancels over the
4M-pixel sums). The target is pre-scaled by 64 on the host (values 0, 64,
128, 192 - exact in bf16), which enables the boundary-count trick below.

Engine budget: the DVE (vector) engine is the bottleneck, so reductions are
fused into compare ops (tensor_scalar with accum_out runs at 4x in bf16)
and the boundary count C is computed entirely on the scalar engine via a
"Relu ladder": yb = 64*t + b is exact in bf16, and
  C_raw[c] = sum(Relu(yb - 64c)) = C[c] + sum_{k>=1} (64k*S[c+k] + C[c+k])
is inverted recursively on the host. The softmax runs in bf16 with one f32
step for the reciprocal (the custom DVE op needs f32 bit layout).
"""

import numpy as np
import ml_dtypes
import concourse.tile as tile
import concourse.mybir as mybir
from concourse import bacc
from concourse.bass_utils import run_bass_kernel_spmd

N_CORES = 8
B, NCLS, H, W = 16, 4, 512, 512
BL = B // N_CORES  # images per core
R = 4  # rows per partition
P = 128
FW = R * W  # free size of one image tile
SMOOTH = 1e-5
TS = 64.0  # target scale factor (class c encoded as 64c)

f32 = mybir.dt.float32
bf16 = mybir.dt.bfloat16
Alu = mybir.AluOpType
AF = mybir.ActivationFunctionType

_cache = {}


def _kernel_body(nc, tc, x_ap, t_ap, sev_ap, idm_ap, out_ap):
    with (
        tc.tile_pool(name="io", bufs=2) as io_pool,
        tc.tile_pool(name="work", bufs=2) as work_pool,
        tc.tile_pool(name="acc", bufs=1) as acc_pool,
        tc.tile_pool(name="ps", bufs=1, space="PSUM") as psum_pool,
    ):
        # one accumulator tile per stat family, so the DVE-written (S, I)
        # and ACT-written (C, Z) accumulators never share a dependency unit
        st_s = acc_pool.tile([P, 2 * NCLS], f32)
        st_c = acc_pool.tile([P, 2 * NCLS], f32)
        st_i = acc_pool.tile([P, 2 * NCLS], f32)
        st_z = acc_pool.tile([P, 2 * NCLS], f32)
        # per-class bias columns for the Relu ladder
        biases = acc_pool.tile([P, NCLS], f32)
        for c in range(NCLS):
            nc.vector.memset(biases[:, c : c + 1], -TS * c)
        # identity matrix for extracting the diagonal of PSUM dot-product blocks
        idm = acc_pool.tile([P, 128], bf16)
        nc.sync.dma_start(idm[:], idm_ap[:])

        for b in range(BL):
            xt = io_pool.tile([P, NCLS, FW], bf16, tag="xt")
            nc.sync.dma_start(xt[:], x_ap[b].rearrange("c (p r) w -> p c (r w)", p=P))
            tt = io_pool.tile([P, FW], bf16, tag="tt")
            nc.sync.dma_start(tt[:], t_ap[b].rearrange("(p r) w -> p (r w)", p=P))
            # halo: hdn[p] = target row 4p+4 (sentinel at p=127 -> image bottom row
            # becomes all-boundary); hup[p] = row 4p-1 (sentinel at p=0).
            hdn = io_pool.tile([P, W], bf16, tag="hdn")
            nc.sync.dma_start(hdn[0 : P - 1, :], t_ap[b, R : H : R, :])
            nc.sync.dma_start(hdn[P - 1 : P, :], sev_ap[:])
            hup = io_pool.tile([P, W], bf16, tag="hup")
            nc.sync.dma_start(hup[1:P, :], t_ap[b, R - 1 : H - 1 : R, :])
            nc.sync.dma_start(hup[0:1, :], sev_ap[:])

            ttf = tt[:]
            tt3 = tt[:].rearrange("p (q w) -> p q w", q=R)

            # vertical edge mask E[r] = (t[r] != t[r+1]), r indexed as (q, w)
            E = work_pool.tile([P, R, W], bf16, tag="E")
            nc.vector.tensor_tensor(
                E[:, 0:3, :], tt3[:, 0:3, :], tt3[:, 1:4, :], op=Alu.not_equal
            )
            nc.vector.tensor_tensor(E[:, 3, :], tt3[:, 3, :], hdn[:], op=Alu.not_equal)

            # boundary mask b = up-differs | down-differs | left | right | border
            bm = work_pool.tile([P, R, W], bf16, tag="bm")
            nc.vector.tensor_tensor(
                bm[:, 1:4, :], E[:, 1:4, :], E[:, 0:3, :], op=Alu.logical_or
            )
            nc.vector.tensor_tensor(bm[:, 0, :], tt3[:, 0, :], hup[:], op=Alu.not_equal)
            nc.vector.tensor_tensor(bm[:, 0, :], bm[:, 0, :], E[:, 0, :], op=Alu.logical_or)

            eh = work_pool.tile([P, FW], bf16, tag="eh")
            nc.vector.tensor_tensor(
                eh[:, 0 : FW - 1], ttf[:, 0 : FW - 1], ttf[:, 1:FW], op=Alu.not_equal
            )
            bfl = bm[:].rearrange("p q w -> p (q w)")
            nc.vector.tensor_tensor(
                bfl[:, 0 : FW - 1], bfl[:, 0 : FW - 1], eh[:, 0 : FW - 1], op=Alu.logical_or
            )
            nc.vector.tensor_tensor(
                bfl[:, 1:FW], bfl[:, 1:FW], eh[:, 0 : FW - 1], op=Alu.logical_or
            )
            nc.vector.memset(bm[:, :, 0:1], 1.0)
            nc.vector.memset(bm[:, :, W - 1 : W], 1.0)

            # softmax probabilities p_c = exp(x_c) / sum_c exp(x_c)
            # (inputs are ~N(0,1); skipping the max-subtraction is safe)
            e = work_pool.tile([P, NCLS, FW], bf16, tag="e")
            for c in range(NCLS):
                nc.scalar.activation(e[:, c], xt[:, c], AF.Exp)
            s01 = work_pool.tile([P, FW], bf16, tag="s01")
            nc.vector.tensor_tensor(s01[:], e[:, 0], e[:, 1], op=Alu.add)
            s23 = work_pool.tile([P, FW], bf16, tag="s23")
            nc.vector.tensor_tensor(s23[:], e[:, 2], e[:, 3], op=Alu.add)
            se = work_pool.tile([P, FW], f32, tag="se")
            nc.vector.tensor_tensor(se[:], s01[:], s23[:], op=Alu.add)
            rcp32 = work_pool.tile([P, FW], f32, tag="rcp32")
            nc.vector.reciprocal_approx_fast(rcp32[:], se[:])
            rcp = work_pool.tile([P, FW], bf16, tag="rcp")
            nc.vector.tensor_scalar(rcp[:], rcp32[:], 0.0, None, op0=Alu.add)
            for c in range(NCLS):
                nc.vector.tensor_tensor(e[:, c], e[:, c], rcp[:], op=Alu.mult)

            # yb = 64*t + b: exact in bf16 ({0,1,64,65,...,193} all representable)
            yb = work_pool.tile([P, FW], bf16, tag="yb")
            nc.vector.tensor_tensor(yb[:], ttf[:], bfl[:], op=Alu.add)

            o = NCLS * b
            junk = work_pool.tile([P, FW], bf16, tag="junk")
            junk128 = work_pool.tile([P, 128], f32, tag="junk128")
            oh = work_pool.tile([P, NCLS, FW], bf16, tag="oh")
            NCH = FW // 128
            for c in range(NCLS):
                # per-(class, family) PSUM tiles: each gets its own bank so
                # extracts never wait on other classes' accumulation groups
                psum_i = psum_pool.tile([P, 128], f32, tag=f"pi{c}")
                psum_z = psum_pool.tile([P, 128], f32, tag=f"pz{c}")
                # oh = (t == 64c); S = row-count fused into the same pass (4x mode)
                nc.vector.tensor_scalar(
                    oh[:, c], ttf[:], TS * c, None, op0=Alu.is_equal, op1=Alu.add,
                    accum_out=st_s[:, o + c : o + c + 1],
                )
                # C ladder: C_raw[c] = sum(Relu(yb - 64c)) on the scalar engine
                nc.scalar.activation(
                    junk[:], yb[:], AF.Relu, bias=biases[:, c : c + 1],
                    accum_out=st_c[:, o + c : o + c + 1],
                )
                # I = sum(p_c*oh_c) and Z = sum(p_c^2) ride the (idle) tensor
                # engine: accumulate 128x128 dot-product blocks in PSUM; the
                # diagonal of the result holds the per-column-chunk sums.
                for ch in range(NCH):
                    sl = slice(ch * 128, (ch + 1) * 128)
                    nc.tensor.matmul(
                        psum_i[:], e[:, c, sl], oh[:, c, sl],
                        start=(ch == 0), stop=(ch == NCH - 1),
                    )
                for ch in range(NCH):
                    sl = slice(ch * 128, (ch + 1) * 128)
                    nc.tensor.matmul(
                        psum_z[:], e[:, c, sl], e[:, c, sl],
                        start=(ch == 0), stop=(ch == NCH - 1),
                    )
                # extract diagonals: I_c = sum(psum_i * idm), Z_c likewise
                nc.vector.scalar_tensor_tensor(
                    out=junk128[:], in0=psum_i[:], scalar=0.0, in1=idm[:],
                    op0=Alu.bypass, op1=Alu.mult,
                    accum_out=st_i[:, o + c : o + c + 1],
                )
                nc.vector.scalar_tensor_tensor(
                    out=junk128[:], in0=psum_z[:], scalar=0.0, in1=idm[:],
                    op0=Alu.bypass, op1=Alu.mult,
                    accum_out=st_z[:, o + c : o + c + 1],
                )

        nc.sync.dma_start(out_ap[:, 0:8], st_s[:])
        nc.sync.dma_start(out_ap[:, 8:16], st_c[:])
        nc.sync.dma_start(out_ap[:, 16:24], st_i[:])
        nc.sync.dma_start(out_ap[:, 24:32], st_z[:])


def _build():
    if "nc" in _cache:
        return _cache["nc"]
    nc = bacc.Bacc("TRN2", target_bir_lowering=False, debug=False, num_devices=N_CORES)
    x_ap = nc.dram_tensor("x", [BL, NCLS, H, W], bf16, kind="ExternalInput").ap()
    t_ap = nc.dram_tensor("t", [BL, H, W], bf16, kind="ExternalInput").ap()
    sev_ap = nc.dram_tensor("sev", [1, W], bf16, kind="ExternalInput").ap()
    idm_ap = nc.dram_tensor("idm", [P, 128], bf16, kind="ExternalInput").ap()
    out_ap = nc.dram_tensor("stats", [P, 2 * 16], f32, kind="ExternalOutput").ap()
    with tile.TileContext(nc) as tc:
        _kernel_body(nc, tc, x_ap, t_ap, sev_ap, idm_ap, out_ap)
    nc.compile()
    _cache["nc"] = nc
    return nc


def _finish(stats_sum):
    """stats_sum: [16] summed raw stats -> scalar loss (host-side epilogue)."""
    s = stats_sum.astype(np.float64)
    S, C_raw, I, Z = s[0:4], s[4:8], s[8:12], s[12:16]
    # invert the Relu ladder: C_raw[c] = C[c] + sum_{k>=1} (64k*S[c+k] + C[c+k])
    C = np.zeros(4)
    for c in range(3, -1, -1):
        C[c] = C_raw[c]
        for k in range(1, 4 - c):
            C[c] -= TS * k * S[c + k] + C[c + k]
    alpha = 1.0 - (C + SMOOTH) / (S + SMOOTH)
    alpha = np.minimum(2.0 * alpha - 1.0, 0.8)
    loss_c = (Z + S - 2.0 * I + SMOOTH) / (Z + S - (1.0 + alpha) * I + SMOOTH)
    return np.float32(loss_c.mean())


def kernel(inputs: np.ndarray, target: np.ndarray) -> np.ndarray:
    nc = _build()
    x = np.ascontiguousarray(inputs.astype(ml_dtypes.bfloat16))
    t = np.ascontiguousarray((target.astype(np.float32) * TS).astype(ml_dtypes.bfloat16))
    sev = np.full((1, W), 7.0 * TS, dtype=ml_dtypes.bfloat16)
    idm = np.eye(P, dtype=ml_dtypes.bfloat16)
    in_maps = [
        {"x": x[c * BL : (c + 1) * BL], "t": t[c * BL : (c + 1) * BL],
         "sev": sev, "idm": idm}
        for c in range(N_CORES)
    ]
    for attempt in range(3):
        res = run_bass_kernel_spmd(nc, in_maps, list(range(N_CORES)))
        stats = np.zeros(16, dtype=np.float64)
        for c in range(N_CORES):
            s = res.results[c]["stats"].astype(np.float64).sum(axis=0)
            g = s.reshape(4, 2, NCLS).sum(axis=1).reshape(-1)  # [S, C_raw, I, Z]
            stats += g
        # S counts must equal the pixel total; retry on transient device faults
        if np.isfinite(stats).all() and abs(stats[0:4].sum() - B * H * W) < 0.5:
            break
    return _finish(stats)


# revision 19
# speedup vs baseline: 1.0137x; 1.0137x over previous
"""BoundaryDoULoss Trainium2 kernel.

Data-parallel over batch: 16 images are sharded 2-per-core across 8
NeuronCores. Each core computes per-class partial sums (S = region count,
C = boundary count, I = sum(probs * onehot), Z = sum(probs^2)) over its
shard; the host reduces the per-class partial scalars and forms alpha and
the final scalar loss.

Layout per image: [512, 512] -> [128 partitions, 4 rows x 512 cols free].
Vertical neighbor comparisons are free-dim shifts within a partition; the
one row per partition that crosses a partition boundary is handled with two
halo tiles DMA-loaded straight from DRAM (rows 4,8,...,508 and 3,7,...,507).
A sentinel row of 448s (not a class id) feeds the image-top/bottom halo
slots so border rows come out as boundary automatically.

Inputs travel as bf16 (halves HBM traffic; quantizing the logits moves the
final loss by ~7e-7 relative - per-pixel rounding noise cancels over the
4M-pixel sums). The target is pre-scaled by 64 on the host (values 0, 64,
128, 192 - exact in bf16), which enables the boundary-count trick below.

Engine budget: the DVE (vector) engine is the bottleneck, so reductions are
fused into compare ops (tensor_scalar with accum_out runs at 4x in bf16)
and the boundary count C is computed entirely on the scalar engine via a
"Relu ladder": yb = 64*t + b is exact in bf16, and
  C_raw[c] = sum(Relu(yb - 64c)) = C[c] + sum_{k>=1} (64k*S[c+k] + C[c+k])
is inverted recursively on the host. The softmax runs in bf16 with one f32
step for the reciprocal (the custom DVE op needs f32 bit layout).
"""

import numpy as np
import ml_dtypes
import concourse.tile as tile
import concourse.mybir as mybir
from concourse import bacc
from concourse.bass_utils import run_bass_kernel_spmd

N_CORES = 8
B, NCLS, H, W = 16, 4, 512, 512
BL = B // N_CORES  # images per core
R = 4  # rows per partition
P = 128
FW = R * W  # free size of one image tile
SMOOTH = 1e-5
TS = 64.0  # target scale factor (class c encoded as 64c)

f32 = mybir.dt.float32
bf16 = mybir.dt.bfloat16
Alu = mybir.AluOpType
AF = mybir.ActivationFunctionType

_cache = {}


def _kernel_body(nc, tc, x_ap, t_ap, sev_ap, idm_ap, out_ap):
    with (
        tc.tile_pool(name="io", bufs=2) as io_pool,
        tc.tile_pool(name="work", bufs=2) as work_pool,
        tc.tile_pool(name="acc", bufs=1) as acc_pool,
        tc.tile_pool(name="ps", bufs=1, space="PSUM") as psum_pool,
    ):
        # one accumulator tile per stat family, so the DVE-written (S, I)
        # and ACT-written (C, Z) accumulators never share a dependency unit
        st_s = acc_pool.tile([P, 2 * NCLS], f32)
        st_c = acc_pool.tile([P, 2 * NCLS], f32)
        st_i = acc_pool.tile([P, 2 * NCLS], f32)
        st_z = acc_pool.tile([P, 2 * NCLS], f32)
        # per-class bias columns for the Relu ladder
        biases = acc_pool.tile([P, NCLS], f32)
        for c in range(NCLS):
            nc.vector.memset(biases[:, c : c + 1], -TS * c)
        # identity matrix for extracting the diagonal of PSUM dot-product blocks
        idm = acc_pool.tile([P, 128], bf16)
        nc.sync.dma_start(idm[:], idm_ap[:])

        for b in range(BL):
            xt = io_pool.tile([P, NCLS, FW], bf16, tag="xt")
            for c in range(NCLS):
                nc.sync.dma_start(
                    xt[:, c], x_ap[b, c].rearrange("(p r) w -> p (r w)", p=P)
                )
            tt = io_pool.tile([P, FW], bf16, tag="tt")
            nc.sync.dma_start(tt[:], t_ap[b].rearrange("(p r) w -> p (r w)", p=P))
            # halo: hdn[p] = target row 4p+4 (sentinel at p=127 -> image bottom row
            # becomes all-boundary); hup[p] = row 4p-1 (sentinel at p=0).
            hdn = io_pool.tile([P, W], bf16, tag="hdn")
            nc.sync.dma_start(hdn[0 : P - 1, :], t_ap[b, R : H : R, :])
            nc.sync.dma_start(hdn[P - 1 : P, :], sev_ap[:])
            hup = io_pool.tile([P, W], bf16, tag="hup")
            nc.sync.dma_start(hup[1:P, :], t_ap[b, R - 1 : H - 1 : R, :])
            nc.sync.dma_start(hup[0:1, :], sev_ap[:])

            ttf = tt[:]
            tt3 = tt[:].rearrange("p (q w) -> p q w", q=R)

            # vertical edge mask E[r] = (t[r] != t[r+1]), r indexed as (q, w)
            E = work_pool.tile([P, R, W], bf16, tag="E")
            nc.vector.tensor_tensor(
                E[:, 0:3, :], tt3[:, 0:3, :], tt3[:, 1:4, :], op=Alu.not_equal
            )
            nc.vector.tensor_tensor(E[:, 3, :], tt3[:, 3, :], hdn[:], op=Alu.not_equal)

            # boundary mask b = up-differs | down-differs | left | right | border
            bm = work_pool.tile([P, R, W], bf16, tag="bm")
            nc.vector.tensor_tensor(
                bm[:, 1:4, :], E[:, 1:4, :], E[:, 0:3, :], op=Alu.logical_or
            )
            nc.vector.tensor_tensor(bm[:, 0, :], tt3[:, 0, :], hup[:], op=Alu.not_equal)
            nc.vector.tensor_tensor(bm[:, 0, :], bm[:, 0, :], E[:, 0, :], op=Alu.logical_or)

            eh = work_pool.tile([P, FW], bf16, tag="eh")
            nc.vector.tensor_tensor(
                eh[:, 0 : FW - 1], ttf[:, 0 : FW - 1], ttf[:, 1:FW], op=Alu.not_equal
            )
            bfl = bm[:].rearrange("p q w -> p (q w)")
            nc.vector.tensor_tensor(
                bfl[:, 0 : FW - 1], bfl[:, 0 : FW - 1], eh[:, 0 : FW - 1], op=Alu.logical_or
            )
            nc.vector.tensor_tensor(
                bfl[:, 1:FW], bfl[:, 1:FW], eh[:, 0 : FW - 1], op=Alu.logical_or
            )
            nc.vector.memset(bm[:, :, 0:1], 1.0)
            nc.vector.memset(bm[:, :, W - 1 : W], 1.0)

            # softmax probabilities p_c = exp(x_c) / sum_c exp(x_c)
            # (inputs are ~N(0,1); skipping the max-subtraction is safe)
            e = work_pool.tile([P, NCLS, FW], bf16, tag="e")
            for c in range(NCLS):
                nc.scalar.activation(e[:, c], xt[:, c], AF.Exp)
            s01 = work_pool.tile([P, FW], bf16, tag="s01")
            nc.vector.tensor_tensor(s01[:], e[:, 0], e[:, 1], op=Alu.add)
            s23 = work_pool.tile([P, FW], bf16, tag="s23")
            nc.vector.tensor_tensor(s23[:], e[:, 2], e[:, 3], op=Alu.add)
            se = work_pool.tile([P, FW], f32, tag="se")
            nc.vector.tensor_tensor(se[:], s01[:], s23[:], op=Alu.add)
            rcp32 = work_pool.tile([P, FW], f32, tag="rcp32")
            nc.vector.reciprocal_approx_fast(rcp32[:], se[:])
            rcp = work_pool.tile([P, FW], bf16, tag="rcp")
            nc.vector.tensor_scalar(rcp[:], rcp32[:], 0.0, None, op0=Alu.add)
            for c in range(NCLS):
                nc.vector.tensor_tensor(e[:, c], e[:, c], rcp[:], op=Alu.mult)

            # yb = 64*t + b: exact in bf16 ({0,1,64,65,...,193} all representable)
            yb = work_pool.tile([P, FW], bf16, tag="yb")
            nc.vector.tensor_tensor(yb[:], ttf[:], bfl[:], op=Alu.add)

            o = NCLS * b
            junk = work_pool.tile([P, FW], bf16, tag="junk")
            junk128 = work_pool.tile([P, 128], f32, tag="junk128")
            oh = work_pool.tile([P, NCLS, FW], bf16, tag="oh")
            NCH = FW // 128
            for c in range(NCLS):
                # per-(class, family) PSUM tiles: each gets its own bank so
                # extracts never wait on other classes' accumulation groups
                psum_i = psum_pool.tile([P, 128], f32, tag=f"pi{c}")
                psum_z = psum_pool.tile([P, 128], f32, tag=f"pz{c}")
                # oh = (t == 64c); S = row-count fused into the same pass (4x mode)
                nc.vector.tensor_scalar(
                    oh[:, c], ttf[:], TS * c, None, op0=Alu.is_equal, op1=Alu.add,
                    accum_out=st_s[:, o + c : o + c + 1],
                )
                # C ladder: C_raw[c] = sum(Relu(yb - 64c)) on the scalar engine
                nc.scalar.activation(
                    junk[:], yb[:], AF.Relu, bias=biases[:, c : c + 1],
                    accum_out=st_c[:, o + c : o + c + 1],
                )
                # I = sum(p_c*oh_c) and Z = sum(p_c^2) ride the (idle) tensor
                # engine: accumulate 128x128 dot-product blocks in PSUM; the
                # diagonal of the result holds the per-column-chunk sums.
                for ch in range(NCH):
                    sl = slice(ch * 128, (ch + 1) * 128)
                    nc.tensor.matmul(
                        psum_i[:], e[:, c, sl], oh[:, c, sl],
                        start=(ch == 0), stop=(ch == NCH - 1),
                    )
                for ch in range(NCH):
                    sl = slice(ch * 128, (ch + 1) * 128)
                    nc.tensor.matmul(
                        psum_z[:], e[:, c, sl], e[:, c, sl],
                        start=(ch == 0), stop=(ch == NCH - 1),
                    )
                # extract diagonals: I_c = sum(psum_i * idm), Z_c likewise
                nc.vector.scalar_tensor_tensor(
                    out=junk128[:], in0=psum_i[:], scalar=0.0, in1=idm[:],
                    op0=Alu.bypass, op1=Alu.mult,
                    accum_out=st_i[:, o + c : o + c + 1],
                )
                nc.vector.scalar_tensor_tensor(
                    out=junk128[:], in0=psum_z[:], scalar=0.0, in1=idm[:],
                    op0=Alu.bypass, op1=Alu.mult,
                    accum_out=st_z[:, o + c : o + c + 1],
                )

        nc.sync.dma_start(out_ap[:, 0:8], st_s[:])
        nc.sync.dma_start(out_ap[:, 8:16], st_c[:])
        nc.sync.dma_start(out_ap[:, 16:24], st_i[:])
        nc.sync.dma_start(out_ap[:, 24:32], st_z[:])


def _build():
    if "nc" in _cache:
        return _cache["nc"]
    nc = bacc.Bacc("TRN2", target_bir_lowering=False, debug=False, num_devices=N_CORES)
    x_ap = nc.dram_tensor("x", [BL, NCLS, H, W], bf16, kind="ExternalInput").ap()
    t_ap = nc.dram_tensor("t", [BL, H, W], bf16, kind="ExternalInput").ap()
    sev_ap = nc.dram_tensor("sev", [1, W], bf16, kind="ExternalInput").ap()
    idm_ap = nc.dram_tensor("idm", [P, 128], bf16, kind="ExternalInput").ap()
    out_ap = nc.dram_tensor("stats", [P, 2 * 16], f32, kind="ExternalOutput").ap()
    with tile.TileContext(nc) as tc:
        _kernel_body(nc, tc, x_ap, t_ap, sev_ap, idm_ap, out_ap)
    nc.compile()
    _cache["nc"] = nc
    return nc


def _finish(stats_sum):
    """stats_sum: [16] summed raw stats -> scalar loss (host-side epilogue)."""
    s = stats_sum.astype(np.float64)
    S, C_raw, I, Z = s[0:4], s[4:8], s[8:12], s[12:16]
    # invert the Relu ladder: C_raw[c] = C[c] + sum_{k>=1} (64k*S[c+k] + C[c+k])
    C = np.zeros(4)
    for c in range(3, -1, -1):
        C[c] = C_raw[c]
        for k in range(1, 4 - c):
            C[c] -= TS * k * S[c + k] + C[c + k]
    alpha = 1.0 - (C + SMOOTH) / (S + SMOOTH)
    alpha = np.minimum(2.0 * alpha - 1.0, 0.8)
    loss_c = (Z + S - 2.0 * I + SMOOTH) / (Z + S - (1.0 + alpha) * I + SMOOTH)
    return np.float32(loss_c.mean())


def kernel(inputs: np.ndarray, target: np.ndarray) -> np.ndarray:
    nc = _build()
    x = np.ascontiguousarray(inputs.astype(ml_dtypes.bfloat16))
    t = np.ascontiguousarray((target.astype(np.float32) * TS).astype(ml_dtypes.bfloat16))
    sev = np.full((1, W), 7.0 * TS, dtype=ml_dtypes.bfloat16)
    idm = np.eye(P, dtype=ml_dtypes.bfloat16)
    in_maps = [
        {"x": x[c * BL : (c + 1) * BL], "t": t[c * BL : (c + 1) * BL],
         "sev": sev, "idm": idm}
        for c in range(N_CORES)
    ]
    for attempt in range(3):
        res = run_bass_kernel_spmd(nc, in_maps, list(range(N_CORES)))
        stats = np.zeros(16, dtype=np.float64)
        for c in range(N_CORES):
            s = res.results[c]["stats"].astype(np.float64).sum(axis=0)
            g = s.reshape(4, 2, NCLS).sum(axis=1).reshape(-1)  # [S, C_raw, I, Z]
            stats += g
        # S counts must equal the pixel total; retry on transient device faults
        if np.isfinite(stats).all() and abs(stats[0:4].sum() - B * H * W) < 0.5:
            break
    return _finish(stats)
